# revision 1
# baseline (speedup 1.0000x reference)
"""Multi-head causal attention (B=2, T=2048, C=1024, H=16) on 8 Trainium2
NeuronCores, tensor-parallel over heads (2 heads per core).

Layout strategy (everything column-major on device, i.e. feature = SBUF
partition dim, token = free dim):
  - host feeds xT [C, B*T] in bf16; per-core w_qkv column slices / w_out row
    slice (bf16).
  - phase 1: QT/KT/VT [128, 4096] = w_c.T @ xT   (accumulate over 8 k-tiles)
  - phase 1.5: PE-transpose V into natural [token, dim] layout, interleaved
    with a ones column per head (row-sums of attention weights come free
    in the AV matmul).
  - phase 2/3 (flash-style, causal tiles skipped): per (batch, q-chunk 512):
      ST[k,q] = KT_tile.T @ QT_chunk  -> +mask on diagonal tiles (DVE)
      PT = exp(ST/8) (ScalarE, PSUM->SBUF, bf16)
      OT_aug[65, q] += Vaug_tile.T @ PT  (row 64 = softmax denominator),
        AV delayed 2 k-tiles behind scores to hide exp latency.
    normalize: sums -> outer-product broadcast (fp32r) -> fast reciprocal
    (DVE, fp32) -> multiply; then
      yT[m-tile, q-chunk] = sum_h wout_h.T @ OT_h  -> direct PSUM->HBM DMA.
  - host: sum 8 partial yT, transpose, add b_out.

Matmuls run in bf16 (fp32 PSUM accumulate); the softmax normalization
chain stays fp32/f32r so per-element output scaling is accurate.
"""

import os
import sys

for _p in ("/opt/trn_rl_repo", "/root/.axon_site/_ro/trn_rl_repo"):
    if os.path.isdir(_p) and _p not in sys.path:
        sys.path.insert(0, _p)

import ml_dtypes
import numpy as np

import concourse.bacc as bacc
import concourse.bass as bass
import concourse.mybir as mybir
import concourse.tile as tile
from concourse.bass_utils import run_bass_kernel_spmd
from concourse.masks import make_identity

B, T, C, H, D = 2, 2048, 1024, 16, 64
NCORES = 8
BT = B * T                      # 4096 flattened tokens
TC = 512                        # token chunk (matmul free dim)
NTC = BT // TC                  # 8 token chunks
FP = mybir.dt.float32
FPR = mybir.dt.float32r
BF = mybir.dt.bfloat16
ACT = mybir.ActivationFunctionType
NEG = -1.0e9
AV_DELAY = 2                    # k-tiles the AV matmul trails the scores

LAST_RESULTS = None             # stashed BassKernelResults for test harness


def build_nc():
    nc = bacc.Bacc(None, target_bir_lowering=False, debug=False)

    xt = nc.declare_dram_parameter("xt", [C, BT], BF, isOutput=False)
    wc = nc.declare_dram_parameter("wc", [C, 384], BF, isOutput=False)
    wout = nc.declare_dram_parameter("wout", [128, C], BF, isOutput=False)
    bqkv = nc.declare_dram_parameter("bqkv", [128, 3], FP, isOutput=False)
    masks = nc.declare_dram_parameter("masks", [512, 512], FP, isOutput=False)
    ones = nc.declare_dram_parameter("ones", [128, 64], BF, isOutput=False)
    onesr = nc.declare_dram_parameter("onesr", [1, 64], FP, isOutput=False)
    yt = nc.declare_dram_parameter("yt", [C, BT], FP, isOutput=True)

    with tile.TileContext(nc) as tc:
        with (
            tc.tile_pool(name="const", bufs=1) as cpool,
            tc.tile_pool(name="big", bufs=1) as bigpool,
            tc.tile_pool(name="sb", bufs=2) as sbpool,
            tc.tile_pool(name="ps", bufs=2, space="PSUM") as pspool,
        ):
            # ---- constants ----
            wc_sb = cpool.tile([128, 8 * 384], BF)      # [cin, k*384 + g*128 + col]
            nc.sync.dma_start(
                out=wc_sb[:].rearrange("b (a c) -> b a c", a=8),
                in_=wc.rearrange("(a b) c -> b a c", a=8),
            )
            # w_out split per local head so the out-projection can contract
            # each head from partition base 0
            wout_sbs = (cpool.tile([64, C], BF, name="wout0"),
                        cpool.tile([64, C], BF, name="wout1"))
            nc.sync.dma_start(out=wout_sbs[0][:], in_=wout[0:64, :])
            nc.sync.dma_start(out=wout_sbs[1][:], in_=wout[64:128, :])
            bq_sb = cpool.tile([128, 3], FP)
            nc.sync.dma_start(out=bq_sb[:], in_=bqkv[:, :])
            masks_sb = cpool.tile([128, 4 * 512], FP)
            nc.sync.dma_start(
                out=masks_sb[:].rearrange("b (a c) -> b a c", a=4),
                in_=masks.rearrange("(a b) c -> b a c", a=4),
            )
            onesr_sb = cpool.tile([1, 64], FPR)
            nc.sync.dma_start(out=onesr_sb[:], in_=onesr.bitcast(FPR)[:, :])
            ident = cpool.tile([128, 128], FP)
            make_identity(nc, ident)

            # ---- persistent intermediates ----
            QT = bigpool.tile([128, BT], BF)
            KT = bigpool.tile([128, BT], BF)
            VT = bigpool.tile([128, BT], FP)
            # V in [token, dim] layout, 130 cols per 128-token block:
            # [V_h0 (64) | ones | V_h1 (64) | ones]
            vaug = bigpool.tile([128, 32 * 130], BF)
            nc.sync.dma_start(
                out=vaug[:].rearrange("p (j a c) -> p j a c", a=2, c=65)[
                    :, :, :, 64:65],
                in_=ones.rearrange("p (j a c) -> p j a c", a=2, c=1)[:, 0:32],
            )

            qkvT = (QT, KT, VT)

            for tcx in range(NTC):
                b, qc = divmod(tcx, 4)
                t0 = tcx * TC

                # ---- phase 1: QKV projection for this token chunk ----
                xts = []
                for k in range(8):
                    xtile = sbpool.tile([128, TC], BF, tag="xt", bufs=10)
                    nc.sync.dma_start(
                        out=xtile[:],
                        in_=xt[k * 128:(k + 1) * 128, t0:t0 + TC],
                    )
                    xts.append(xtile)
                for g in range(3):
                    ps = pspool.tile([128, TC], FP, tag="q", bufs=2)
                    for k in range(8):
                        nc.tensor.matmul(
                            ps[:],
                            wc_sb[:, k * 384 + g * 128:k * 384 + (g + 1) * 128],
                            xts[k][:],
                            start=(k == 0),
                            stop=(k == 7),
                        )
                    nc.scalar.activation(
                        qkvT[g][:, t0:t0 + TC], ps[:], ACT.Identity,
                        bias=bq_sb[:, g:g + 1],
                    )

                # ---- phase 1.5: transpose this chunk's V into vaug ----
                for j in range(4):
                    jj = tcx * 4 + j
                    tp = pspool.tile([128, 128], FP, tag="q", bufs=2, name="tp")
                    nc.tensor.transpose(
                        tp[:], VT[:, jj * 128:(jj + 1) * 128], ident[:]
                    )
                    nc.vector.tensor_copy(
                        vaug[:].rearrange("p (j a c) -> p j a c", a=2, c=65)[
                            :, jj, :, 0:64],
                        tp[:].rearrange("p (a c) -> p a c", c=64),
                    )

                # ---- phase 2/3: causal attention for (b, qc) ----
                n_kt = 4 * (qc + 1)
                otps = [
                    pspool.tile([65, TC], FP, tag="av", bufs=2, name=f"otp{_h}")
                    for _h in range(2)
                ]
                pts = {}

                def emit_av(j, kg0):
                    for h in range(2):
                        nc.tensor.matmul(
                            otps[h][:],
                            vaug[:, kg0 * 130 + h * 65:kg0 * 130 + h * 65 + 65],
                            pts.pop((j, h))[:],
                            start=(j == 0), stop=(j == n_kt - 1),
                            skip_group_check=True,
                        )

                for kt in range(n_kt):
                    kg = b * 16 + kt
                    for h in range(2):
                        sp = pspool.tile([128, TC], FP, tag="s", bufs=2)
                        nc.tensor.matmul(
                            sp[:],
                            KT[h * 64:(h + 1) * 64, kg * 128:(kg + 1) * 128],
                            QT[h * 64:(h + 1) * 64, t0:t0 + TC],
                            start=True, stop=True,
                        )
                        if kt >= 4 * qc:
                            v = kt - 4 * qc
                            nc.vector.tensor_add(
                                sp[:], sp[:], masks_sb[:, v * 512:(v + 1) * 512]
                            )
                        pt = sbpool.tile([128, TC], BF, tag="pt",
                                         bufs=2 * (AV_DELAY + 1))
                        nc.scalar.activation(pt[:], sp[:], ACT.Exp, scale=0.125)
                        pts[(kt, h)] = pt
                    if kt >= AV_DELAY:
                        emit_av(kt - AV_DELAY, b * 16 + kt - AV_DELAY)
                for j in range(max(n_kt - AV_DELAY, 0), n_kt):
                    emit_av(j, b * 16 + j)

                # ---- normalize: 1/rowsum broadcast, per head ----
                ots = []
                for h in range(2):
                    rcsum = sbpool.tile([1, TC], FPR, tag=f"rc{h}", bufs=2,
                                        name=f"rc{h}")
                    with nc.allow_low_precision(reason="softmax sums f32r"):
                        nc.scalar.copy(rcsum[:], otps[h][64:65, :])
                    bch = pspool.tile([64, TC], FP, tag="s", bufs=2,
                                      name=f"bc{h}")
                    nc.tensor.matmul(bch[:], onesr_sb[0:1, :], rcsum[:],
                                     start=True, stop=True)
                    bcs = sbpool.tile([64, TC], FP, tag=f"bcs{h}", bufs=2,
                                      name=f"bcs{h}")
                    nc.vector.reciprocal_approx_fast(out=bcs[:], in_=bch[:])
                    oth = sbpool.tile([64, TC], BF, tag=f"ot{h}", bufs=2,
                                      name=f"ot{h}")
                    nc.vector.tensor_mul(oth[:], otps[h][0:64, :], bcs[:])
                    ots.append(oth)

                # ---- phase 4: output projection (contract heads) ----
                for m in range(8):
                    yp = pspool.tile([128, TC], FP, tag="y", bufs=2)
                    for h in range(2):
                        nc.tensor.matmul(
                            yp[:],
                            wout_sbs[h][:, m * 128:(m + 1) * 128],
                            ots[h][:],
                            start=(h == 0), stop=(h == 1),
                        )
                    ys = sbpool.tile([128, TC], FP, tag="ys", bufs=4)
                    if m % 2 == 0:
                        nc.scalar.copy(ys[:], yp[:])
                    else:
                        nc.vector.tensor_copy(ys[:], yp[:])
                    nc.sync.dma_start(
                        out=yt[m * 128:(m + 1) * 128, t0:t0 + TC], in_=ys[:]
                    )
    nc.compile()
    return nc


def make_in_maps(x, w_qkv, b_qkv):
    x = np.ascontiguousarray(np.asarray(x, np.float32).reshape(BT, C))
    xT = np.ascontiguousarray(x.T).astype(ml_dtypes.bfloat16)
    w_qkv = np.asarray(w_qkv, np.float32)
    b_qkv = np.asarray(b_qkv, np.float32)

    mask = np.empty((512, 512), np.float32)
    for v in range(4):
        kk = np.arange(128)[:, None] + 128 * v
        qq = np.arange(512)[None, :]
        mask[v * 128:(v + 1) * 128] = np.where(kk <= qq, 0.0, NEG)

    in_maps = []
    for c in range(NCORES):
        sl = slice(c * 128, (c + 1) * 128)
        wcs = np.concatenate(
            [w_qkv[:, sl], w_qkv[:, 1024:][:, sl], w_qkv[:, 2048:][:, sl]], axis=1
        )
        bq = np.stack(
            [b_qkv[sl], b_qkv[1024:][sl], b_qkv[2048:][sl]], axis=1
        )
        in_maps.append({
            "xt": xT,
            "wc": np.ascontiguousarray(wcs).astype(ml_dtypes.bfloat16),
            "wout": None,  # filled by caller (needs w_out)
            "bqkv": np.ascontiguousarray(bq),
            "masks": mask,
            "ones": np.ones((128, 64), ml_dtypes.bfloat16),
            "onesr": np.ones((1, 64), np.float32),
        })
    return in_maps


_NC_CACHE = None


def kernel(x, w_qkv, b_qkv, w_out, b_out):
    global _NC_CACHE, LAST_RESULTS
    if _NC_CACHE is None:
        _NC_CACHE = build_nc()
    nc = _NC_CACHE

    w_out = np.asarray(w_out, np.float32)
    in_maps = make_in_maps(x, w_qkv, b_qkv)
    for c in range(NCORES):
        in_maps[c]["wout"] = np.ascontiguousarray(
            w_out[c * 128:(c + 1) * 128, :]).astype(ml_dtypes.bfloat16)

    res = run_bass_kernel_spmd(
        nc, in_maps, list(range(NCORES)),
        trace=bool(os.environ.get("BASS_TRACE")),
    )
    LAST_RESULTS = res

    acc = np.zeros((C, BT), np.float64)
    for out_map in res.results:
        acc += out_map["yt"].astype(np.float64)
    y = acc.T.astype(np.float32) + np.asarray(b_out, np.float32)[None, :]
    return y.reshape(B, T, C)



# revision 11
# speedup vs baseline: 1.0113x; 1.0113x over previous
"""Multi-head causal attention (B=2, T=2048, C=1024, H=16) on 8 Trainium2
NeuronCores, tensor-parallel over heads (2 heads per core).

v2 — engine-balanced redesign of the v1 flash kernel:
  - scores: the two heads' K^T@Q matmuls are issued adjacently with base
    partitions 0/64 so they land on different PE row groups and run
    CONCURRENTLY (row tiling), into the two halves of one [128,1024] PSUM
    pair tile.
  - causal mask: added on the PE as a tiny N=128 matmul (identity @ mtri)
    accumulated into the diagonal 128-block of the scores group — replaces
    the expensive [128,512] DVE adds.
  - exp: one ScalarE ACTIVATE per k-tile covering BOTH heads via a 3D AP
    over the [128,1024] pair (halves ScalarE instruction count).
  - diagonal k-tiles only compute the valid q-range (scores, exp, AV all
    use free dim 512-128*v).
  - AV keeps the ones-column trick (stationary [128,65] Vaug) for free
    softmax denominators; both heads' O accumulate across the whole chunk
    in two single-buffered PSUM banks.
  - normalize: rowsums -> reciprocal on [2,512] -> one fp32r broadcast
    matmul -> ScalarE copy -> two DVE muls into a combined ots[128,512]
    (h0 on partitions 0-63, h1 on 64-127).
  - out-projection: single full-contract matmul per m-tile (contract over
    both heads at once), DVE copy to bf16, DMA out.
  - x input host-packed as [p, chunk, ktile, t] so each chunk's load is one
    DMA with 8KB contiguous lines; y output in bf16 (halves write traffic).
"""

import os
import sys

for _p in ("/opt/trn_rl_repo", "/root/.axon_site/_ro/trn_rl_repo"):
    if os.path.isdir(_p) and _p not in sys.path:
        sys.path.insert(0, _p)

import ml_dtypes
import numpy as np

import concourse.bacc as bacc
import concourse.bass as bass
import concourse.mybir as mybir
import concourse.tile as tile
from concourse.bass_utils import run_bass_kernel_spmd
from concourse.masks import make_identity

B, T, C, H, D = 2, 2048, 1024, 16, 64
NCORES = 8
BT = B * T                      # 4096 flattened tokens
TC = 512                        # token chunk (matmul free dim)
NTC = BT // TC                  # 8 token chunks
FP = mybir.dt.float32
FPR = mybir.dt.float32r
BF = mybir.dt.bfloat16
ACT = mybir.ActivationFunctionType
NEG = -1.0e9
AV_DELAY = 2                    # k-tiles the AV matmul trails the scores

LAST_RESULTS = None             # stashed BassKernelResults for test harness


def build_nc():
    nc = bacc.Bacc(None, target_bir_lowering=False, debug=False)

    xh = nc.declare_dram_parameter("xh", [128, NTC * 4096], BF, isOutput=False)
    wc = nc.declare_dram_parameter("wc", [C, 384], BF, isOutput=False)
    wout = nc.declare_dram_parameter("wout", [128, C], BF, isOutput=False)
    bqkv = nc.declare_dram_parameter("bqkv", [128, 3], FP, isOutput=False)
    mtri = nc.declare_dram_parameter("mtri", [128, 128], BF, isOutput=False)
    ones = nc.declare_dram_parameter("ones", [128, 64], BF, isOutput=False)
    onesr = nc.declare_dram_parameter("onesr", [1, 64], FP, isOutput=False)
    yh = nc.declare_dram_parameter("yh", [128, NTC * 4096], BF, isOutput=True)

    with tile.TileContext(nc) as tc:
        with (
            tc.tile_pool(name="const", bufs=1) as cpool,
            tc.tile_pool(name="big", bufs=1) as bigpool,
            tc.tile_pool(name="sb", bufs=2) as sbpool,
            tc.tile_pool(name="ps", bufs=2, space="PSUM") as pspool,
        ):
            # ---- constants ----
            wc_sb = cpool.tile([128, 8 * 384], BF)      # [cin, k*384 + g*128 + col]
            nc.sync.dma_start(
                out=wc_sb[:].rearrange("b (a c) -> b a c", a=8),
                in_=wc.rearrange("(a b) c -> b a c", a=8),
            )
            wout_sb = cpool.tile([128, C], BF)          # rows: h0 d0-63 | h1 d0-63
            nc.sync.dma_start(out=wout_sb[:], in_=wout[:, :])
            bq_sb = cpool.tile([128, 3], FP)
            nc.sync.dma_start(out=bq_sb[:], in_=bqkv[:, :])
            mtri_sb = cpool.tile([128, 128], BF)
            nc.sync.dma_start(out=mtri_sb[:], in_=mtri[:, :])
            onesr_sb = cpool.tile([1, 64], FPR)
            nc.sync.dma_start(out=onesr_sb[:], in_=onesr.bitcast(FPR)[:, :])
            ident = cpool.tile([128, 128], BF)
            make_identity(nc, ident)

            # ---- persistent intermediates ----
            QT = bigpool.tile([128, BT], BF)
            KT = bigpool.tile([128, BT], BF)
            VT = bigpool.tile([128, BT], BF)
            # V in [token, dim] layout, 130 cols per 128-token block:
            # [V_h0 (64) | ones | V_h1 (64) | ones]
            vaug = bigpool.tile([128, 32 * 130], BF)
            nc.sync.dma_start(
                out=vaug[:].rearrange("p (j a c) -> p j a c", a=2, c=65)[
                    :, :, :, 64:65],
                in_=ones.rearrange("p (j a c) -> p j a c", a=2, c=1)[:, 0:32],
            )

            qkvT = (QT, KT, VT)

            for tcx in range(NTC):
                b, qc = divmod(tcx, 4)
                t0 = tcx * TC

                # ---- phase 1: QKV projection for this token chunk ----
                xtile = sbpool.tile([128, 4096], BF, tag="xt", bufs=3)
                nc.sync.dma_start(
                    out=xtile[:], in_=xh[:, tcx * 4096:(tcx + 1) * 4096]
                )
                for g in range(3):
                    qp = pspool.tile([128, TC], FP, tag="t", bufs=2, name="qp")
                    for k in range(8):
                        nc.tensor.matmul(
                            qp[:],
                            wc_sb[:, k * 384 + g * 128:k * 384 + (g + 1) * 128],
                            xtile[:, k * TC:(k + 1) * TC],
                            start=(k == 0),
                            stop=(k == 7),
                        )
                    nc.scalar.activation(
                        qkvT[g][:, t0:t0 + TC], qp[:], ACT.Identity,
                        bias=bq_sb[:, g:g + 1],
                    )

                # ---- phase 1.5: transpose this chunk's V into vaug ----
                for j in range(4):
                    jj = tcx * 4 + j
                    tpf = pspool.tile([128, TC], FP, tag="t", bufs=2, name="tp")
                    tp = tpf.bitcast(BF)[:, 0:128]
                    nc.tensor.transpose(
                        tp, VT[:, jj * 128:(jj + 1) * 128], ident[:]
                    )
                    nc.vector.tensor_copy(
                        vaug[:].rearrange("p (j a c) -> p j a c", a=2, c=65)[
                            :, jj, :, 0:64],
                        tp.rearrange("p (a c) -> p a c", c=64),
                    )

                # ---- phase 2/3: causal attention for (b, qc) ----
                n_kt = 4 * (qc + 1)
                otp = pspool.tile([65, 2 * TC], FP, tag="o", bufs=1,
                                  name="otp")
                pts = {}

                def emit_av(j, kg0):
                    pt, qs = pts.pop(j)
                    for h in range(2):
                        nc.tensor.matmul(
                            otp[:, h * TC + qs:(h + 1) * TC],
                            vaug[:, kg0 * 130 + h * 65:kg0 * 130 + h * 65 + 65],
                            pt[:, h * TC + qs:(h + 1) * TC],
                            start=(j == 0), stop=(j == n_kt - 1),
                            skip_group_check=True,
                        )

                for kt in range(n_kt):
                    kg = b * 16 + kt
                    diag = kt >= 4 * qc
                    v = kt - 4 * qc if diag else 0
                    qs = v * 128
                    sp = pspool.tile([128, 2 * TC], FP, tag="s", bufs=2,
                                     name="sp")
                    for h in range(2):
                        nc.tensor.matmul(
                            sp[:, h * TC + qs:(h + 1) * TC],
                            KT[h * 64:(h + 1) * 64, kg * 128:(kg + 1) * 128],
                            QT[h * 64:(h + 1) * 64, t0 + qs:t0 + TC],
                            start=True, stop=not diag,
                            skip_group_check=True,
                        )
                    if diag:
                        for h in range(2):
                            nc.tensor.matmul(
                                sp[:, h * TC + qs:h * TC + qs + 128],
                                ident[:],
                                mtri_sb[:],
                                start=False, stop=True,
                                skip_group_check=True,
                            )
                    pt = sbpool.tile([128, 2 * TC], BF, tag="pt", bufs=5,
                                     name="pt")
                    nc.scalar.activation(
                        pt[:].rearrange("p (j q) -> p j q", j=2)[:, :, qs:TC],
                        sp[:].rearrange("p (j q) -> p j q", j=2)[:, :, qs:TC],
                        ACT.Exp, scale=0.125,
                    )
                    pts[kt] = (pt, qs)
                    if kt >= AV_DELAY:
                        emit_av(kt - AV_DELAY, b * 16 + kt - AV_DELAY)
                for j in range(max(n_kt - AV_DELAY, 0), n_kt):
                    emit_av(j, b * 16 + j)

                # ---- normalize: 1/rowsum, DMA partition-broadcast ----
                rc2 = sbpool.tile([1, 2 * TC], FP, tag="rc", bufs=2,
                                  name="rc2")
                nc.scalar.copy(rc2[:], otp[64:65, :])
                rinv = sbpool.tile([1, 2 * TC], FP, tag="ri", bufs=2,
                                   name="rinv")
                nc.vector.reciprocal_approx_fast(out=rinv[:], in_=rc2[:])
                bcs = sbpool.tile([128, 2 * TC], FP, tag="bc", bufs=2,
                                  name="bcs")
                nc.gpsimd.partition_broadcast(out_ap=bcs[:], in_ap=rinv[:])
                ots = sbpool.tile([128, TC], BF, tag="ot", bufs=2, name="ots")
                nc.vector.tensor_mul(ots[0:64, :], otp[0:64, 0:TC],
                                     bcs[0:64, 0:TC])
                nc.vector.tensor_mul(ots[64:128, :], otp[0:64, TC:2 * TC],
                                     bcs[64:128, TC:2 * TC])

                # ---- phase 4: output projection (contract both heads) ----
                for m in range(8):
                    yp = pspool.tile([128, TC], FP, tag="t", bufs=2, name="yp")
                    nc.tensor.matmul(
                        yp[:], wout_sb[:, m * 128:(m + 1) * 128], ots[:],
                        start=True, stop=True,
                    )
                    ysb = sbpool.tile([128, TC], BF, tag="ys", bufs=4,
                                      name="ysb")
                    nc.vector.tensor_copy(ysb[:], yp[:])
                    nc.sync.dma_start(
                        out=yh[:, tcx * 4096 + m * TC:tcx * 4096 + (m + 1) * TC],
                        in_=ysb[:],
                    )
    nc.compile()
    return nc


def make_in_maps(x, w_qkv, b_qkv, w_out):
    x = np.ascontiguousarray(np.asarray(x, np.float32).reshape(BT, C))
    xT = np.ascontiguousarray(x.T)                    # [C, BT]
    # [a(8), p(128), tcx(8), t(512)] -> [p, tcx, a, t]
    xhp = np.ascontiguousarray(
        xT.reshape(8, 128, NTC, TC).transpose(1, 2, 0, 3).reshape(128, -1)
    ).astype(ml_dtypes.bfloat16)
    w_qkv = np.asarray(w_qkv, np.float32)
    b_qkv = np.asarray(b_qkv, np.float32)
    w_out = np.asarray(w_out, np.float32)

    kk = np.arange(128)[:, None]
    qq = np.arange(128)[None, :]
    mtri = np.where(kk <= qq, 0.0, NEG).astype(ml_dtypes.bfloat16)

    in_maps = []
    for c in range(NCORES):
        sl = slice(c * 128, (c + 1) * 128)
        wcs = np.concatenate(
            [w_qkv[:, sl], w_qkv[:, 1024:][:, sl], w_qkv[:, 2048:][:, sl]],
            axis=1,
        )
        bq = np.stack(
            [b_qkv[sl], b_qkv[1024:][sl], b_qkv[2048:][sl]], axis=1
        )
        in_maps.append({
            "xh": xhp,
            "wc": np.ascontiguousarray(wcs).astype(ml_dtypes.bfloat16),
            "wout": np.ascontiguousarray(w_out[sl, :]).astype(
                ml_dtypes.bfloat16),
            "bqkv": np.ascontiguousarray(bq),
            "mtri": mtri,
            "ones": np.ones((128, 64), ml_dtypes.bfloat16),
            "onesr": np.ones((1, 64), np.float32),
        })
    return in_maps


_NC_CACHE = None


def kernel(x, w_qkv, b_qkv, w_out, b_out):
    global _NC_CACHE, LAST_RESULTS
    if _NC_CACHE is None:
        _NC_CACHE = build_nc()
    nc = _NC_CACHE

    in_maps = make_in_maps(x, w_qkv, b_qkv, w_out)

    res = run_bass_kernel_spmd(
        nc, in_maps, list(range(NCORES)),
        trace=bool(os.environ.get("BASS_TRACE")),
    )
    LAST_RESULTS = res

    acc = np.zeros((C, BT), np.float32)
    for out_map in res.results:
        # yh [p, tcx(8), m(8), t(512)] -> [m, p, tcx, t] -> [C, BT]
        yc = np.asarray(out_map["yh"]).reshape(128, NTC, 8, TC)
        acc += yc.transpose(2, 0, 1, 3).reshape(C, BT).astype(np.float32)
    y = acc.T + np.asarray(b_out, np.float32)[None, :]
    return y.reshape(B, T, C)


# revision 16
# speedup vs baseline: 1.4216x; 1.4056x over previous
"""Multi-head causal attention (B=2, T=2048, C=1024, H=16) on 8 Trainium2
NeuronCores, tensor-parallel over heads (2 heads per core).

v2 — engine-balanced redesign of the v1 flash kernel:
  - scores: the two heads' K^T@Q matmuls are issued adjacently with base
    partitions 0/64 so they land on different PE row groups and run
    CONCURRENTLY (row tiling), into the two halves of one [128,1024] PSUM
    pair tile.
  - causal mask: added on the PE as a tiny N=128 matmul (identity @ mtri)
    accumulated into the diagonal 128-block of the scores group — replaces
    the expensive [128,512] DVE adds.
  - exp: one ScalarE ACTIVATE per k-tile covering BOTH heads via a 3D AP
    over the [128,1024] pair (halves ScalarE instruction count).
  - diagonal k-tiles only compute the valid q-range (scores, exp, AV all
    use free dim 512-128*v).
  - AV keeps the ones-column trick (stationary [128,65] Vaug) for free
    softmax denominators; both heads' O accumulate across the whole chunk
    in two single-buffered PSUM banks.
  - normalize: rowsums -> reciprocal on [2,512] -> one fp32r broadcast
    matmul -> ScalarE copy -> two DVE muls into a combined ots[128,512]
    (h0 on partitions 0-63, h1 on 64-127).
  - out-projection: single full-contract matmul per m-tile (contract over
    both heads at once), DVE copy to bf16, DMA out.
  - x input host-packed as [p, chunk, ktile, t] so each chunk's load is one
    DMA with 8KB contiguous lines; y output in bf16 (halves write traffic).
"""

import os
import sys

for _p in ("/opt/trn_rl_repo", "/root/.axon_site/_ro/trn_rl_repo"):
    if os.path.isdir(_p) and _p not in sys.path:
        sys.path.insert(0, _p)

import ml_dtypes
import numpy as np

import concourse.bacc as bacc
import concourse.bass as bass
import concourse.mybir as mybir
import concourse.tile as tile
from concourse.bass_utils import run_bass_kernel_spmd
from concourse.masks import make_identity

B, T, C, H, D = 2, 2048, 1024, 16, 64
NCORES = 8
BT = B * T                      # 4096 flattened tokens
TC = 512                        # token chunk (matmul free dim)
NTC = BT // TC                  # 8 token chunks
FP = mybir.dt.float32
FPR = mybir.dt.float32r
BF = mybir.dt.bfloat16
ACT = mybir.ActivationFunctionType
NEG = -1.0e9
AV_DELAY = 2                    # k-tiles the AV matmul trails the scores

LAST_RESULTS = None             # stashed BassKernelResults for test harness


def build_nc():
    nc = bacc.Bacc(None, target_bir_lowering=False, debug=False)

    xh = nc.declare_dram_parameter("xh", [128, NTC * 4096], BF, isOutput=False)
    wc = nc.declare_dram_parameter("wc", [C, 384], BF, isOutput=False)
    wout = nc.declare_dram_parameter("wout", [128, C], BF, isOutput=False)
    bqkv = nc.declare_dram_parameter("bqkv", [128, 3], FP, isOutput=False)
    mtri = nc.declare_dram_parameter("mtri", [128, 128], BF, isOutput=False)
    ones = nc.declare_dram_parameter("ones", [128, 64], BF, isOutput=False)
    onesr = nc.declare_dram_parameter("onesr", [1, 64], FP, isOutput=False)
    yh = nc.declare_dram_parameter("yh", [128, NTC * 4096], BF, isOutput=True)

    with tile.TileContext(nc) as tc:
        with (
            tc.tile_pool(name="const", bufs=1) as cpool,
            tc.tile_pool(name="big", bufs=1) as bigpool,
            tc.tile_pool(name="sb", bufs=2) as sbpool,
            tc.tile_pool(name="ps", bufs=2, space="PSUM") as pspool,
        ):
            # ---- constants ----
            wc_sb = cpool.tile([128, 8 * 384], BF)      # [cin, k*384 + g*128 + col]
            nc.sync.dma_start(
                out=wc_sb[:].rearrange("b (a c) -> b a c", a=8),
                in_=wc.rearrange("(a b) c -> b a c", a=8),
            )
            wout_sb = cpool.tile([128, C], BF)          # rows: h0 d0-63 | h1 d0-63
            nc.sync.dma_start(out=wout_sb[:], in_=wout[:, :])
            bq_sb = cpool.tile([128, 3], FP)
            nc.sync.dma_start(out=bq_sb[:], in_=bqkv[:, :])
            mtri_sb = cpool.tile([128, 128], BF)
            nc.sync.dma_start(out=mtri_sb[:], in_=mtri[:, :])
            onesr_sb = cpool.tile([1, 64], FPR)
            nc.sync.dma_start(out=onesr_sb[:], in_=onesr.bitcast(FPR)[:, :])
            ident = cpool.tile([128, 128], BF)
            make_identity(nc, ident)

            # ---- persistent intermediates ----
            QT = bigpool.tile([128, BT], BF)
            KT = bigpool.tile([128, BT], BF)
            VT = bigpool.tile([128, BT], BF)
            # V in [token, dim] layout, 130 cols per 128-token block:
            # [V_h0 (64) | ones | V_h1 (64) | ones]
            vaug = bigpool.tile([128, 32 * 130], BF)
            ones_sb = cpool.tile([128, 64], BF)
            nc.sync.dma_start(out=ones_sb[:], in_=ones[:, :])
            nc.vector.tensor_copy(
                vaug[:].rearrange("p (j a c) -> p j a c", a=2, c=65)[
                    :, :, :, 64:65],
                ones_sb[:].rearrange("p (j a c) -> p j a c", a=2, c=1)[:, 0:32],
            )

            qkvT = (QT, KT, VT)

            def emit_outproj(ots_prev, t0_prev):
                for m in range(8):
                    yp = pspool.tile([128, TC], FP, tag="t", bufs=2,
                                     name="yp")
                    nc.tensor.matmul(
                        yp[:], wout_sb[:, m * 128:(m + 1) * 128],
                        ots_prev[:], start=True, stop=True,
                    )
                    ysb = sbpool.tile([128, TC], BF, tag="ys", bufs=4,
                                      name="ysb")
                    nc.vector.tensor_copy(ysb[:], yp[:])
                    nc.sync.dma_start(
                        out=yh[:, (t0_prev // TC) * 4096
                               + m * TC:(t0_prev // TC) * 4096
                               + (m + 1) * TC],
                        in_=ysb[:],
                    )

            pending = None          # (ots, t0) of the previous chunk

            for tcx in range(NTC):
                b, qc = divmod(tcx, 4)
                t0 = tcx * TC

                # ---- phase 1: QKV projection for this token chunk ----
                xtile = sbpool.tile([128, 4096], BF, tag="xt", bufs=3)
                nc.sync.dma_start(
                    out=xtile[:], in_=xh[:, tcx * 4096:(tcx + 1) * 4096]
                )
                for g in range(3):
                    qp = pspool.tile([128, TC], FP, tag="t", bufs=2, name="qp")
                    for k in range(8):
                        nc.tensor.matmul(
                            qp[:],
                            wc_sb[:, k * 384 + g * 128:k * 384 + (g + 1) * 128],
                            xtile[:, k * TC:(k + 1) * TC],
                            start=(k == 0),
                            stop=(k == 7),
                        )
                    nc.vector.tensor_scalar_add(
                        qkvT[g][:, t0:t0 + TC], qp[:], bq_sb[:, g:g + 1],
                    )

                # ---- phase 1.5: transpose this chunk's V into vaug ----
                for j in range(4):
                    jj = tcx * 4 + j
                    tpf = pspool.tile([128, TC], FP, tag="t", bufs=2, name="tp")
                    tp = tpf.bitcast(BF)[:, 0:128]
                    nc.tensor.transpose(
                        tp, VT[:, jj * 128:(jj + 1) * 128], ident[:]
                    )
                    nc.vector.tensor_copy(
                        vaug[:].rearrange("p (j a c) -> p j a c", a=2, c=65)[
                            :, jj, :, 0:64],
                        tp.rearrange("p (a c) -> p a c", c=64),
                    )

                # ---- phase 2/3: causal attention for (b, qc) ----
                n_kt = 4 * (qc + 1)
                otp = pspool.tile([65, 2 * TC], FP, tag="o", bufs=1,
                                  name="otp")
                pts = {}

                def emit_av(j, kg0):
                    pt, qs = pts.pop(j)
                    for h in range(2):
                        nc.tensor.matmul(
                            otp[:, h * TC + qs:(h + 1) * TC],
                            vaug[:, kg0 * 130 + h * 65:kg0 * 130 + h * 65 + 65],
                            pt[:, h * TC + qs:(h + 1) * TC],
                            start=(j == 0), stop=(j == n_kt - 1),
                            skip_group_check=True,
                        )

                for kt in range(n_kt):
                    kg = b * 16 + kt
                    diag = kt >= 4 * qc
                    v = kt - 4 * qc if diag else 0
                    qs = v * 128
                    sp = pspool.tile([128, 2 * TC], FP, tag="s", bufs=2,
                                     name="sp")
                    for h in range(2):
                        nc.tensor.matmul(
                            sp[:, h * TC + qs:(h + 1) * TC],
                            KT[h * 64:(h + 1) * 64, kg * 128:(kg + 1) * 128],
                            QT[h * 64:(h + 1) * 64, t0 + qs:t0 + TC],
                            start=True, stop=not diag,
                            skip_group_check=True,
                        )
                    if diag:
                        for h in range(2):
                            nc.tensor.matmul(
                                sp[:, h * TC + qs:h * TC + qs + 128],
                                ident[:],
                                mtri_sb[:],
                                start=False, stop=True,
                                skip_group_check=True,
                            )
                    pt = sbpool.tile([128, 2 * TC], BF, tag="pt", bufs=5,
                                     name="pt")
                    nc.scalar.activation(
                        pt[:].rearrange("p (j q) -> p j q", j=2)[:, :, qs:TC],
                        sp[:].rearrange("p (j q) -> p j q", j=2)[:, :, qs:TC],
                        ACT.Exp, scale=0.125,
                    )
                    pts[kt] = (pt, qs)
                    if kt == 1 and pending is not None:
                        # previous chunk's out-projection, delayed so the
                        # PE queue has fill work while its normalize chain
                        # (ScalarE->DVE->GpSimd->DVE) completes
                        emit_outproj(*pending)
                        pending = None
                    if kt >= AV_DELAY:
                        emit_av(kt - AV_DELAY, b * 16 + kt - AV_DELAY)
                for j in range(max(n_kt - AV_DELAY, 0), n_kt):
                    emit_av(j, b * 16 + j)

                # ---- normalize: 1/rowsum, DMA partition-broadcast ----
                rc2 = sbpool.tile([1, 2 * TC], FP, tag="rc", bufs=2,
                                  name="rc2")
                nc.scalar.copy(rc2[:], otp[64:65, :])
                rinv = sbpool.tile([1, 2 * TC], FP, tag="ri", bufs=2,
                                   name="rinv")
                nc.vector.reciprocal_approx_fast(out=rinv[:], in_=rc2[:])
                bcs = sbpool.tile([128, 2 * TC], FP, tag="bc", bufs=2,
                                  name="bcs")
                nc.gpsimd.partition_broadcast(out_ap=bcs[:], in_ap=rinv[:])
                ots = sbpool.tile([128, TC], BF, tag="ot", bufs=2, name="ots")
                nc.vector.tensor_mul(ots[0:64, :], otp[0:64, 0:TC],
                                     bcs[0:64, 0:TC])
                nc.vector.tensor_mul(ots[64:128, :], otp[0:64, TC:2 * TC],
                                     bcs[64:128, TC:2 * TC])
                pending = (ots, t0)

            emit_outproj(*pending)
    nc.compile()
    return nc


def make_in_maps(x, w_qkv, b_qkv, w_out):
    x = np.ascontiguousarray(np.asarray(x, np.float32).reshape(BT, C))
    xT = np.ascontiguousarray(x.T)                    # [C, BT]
    # [a(8), p(128), tcx(8), t(512)] -> [p, tcx, a, t]
    xhp = np.ascontiguousarray(
        xT.reshape(8, 128, NTC, TC).transpose(1, 2, 0, 3).reshape(128, -1)
    ).astype(ml_dtypes.bfloat16)
    w_qkv = np.asarray(w_qkv, np.float32)
    b_qkv = np.asarray(b_qkv, np.float32)
    w_out = np.asarray(w_out, np.float32)

    kk = np.arange(128)[:, None]
    qq = np.arange(128)[None, :]
    mtri = np.where(kk <= qq, 0.0, NEG).astype(ml_dtypes.bfloat16)

    in_maps = []
    for c in range(NCORES):
        sl = slice(c * 128, (c + 1) * 128)
        wcs = np.concatenate(
            [w_qkv[:, sl], w_qkv[:, 1024:][:, sl], w_qkv[:, 2048:][:, sl]],
            axis=1,
        )
        bq = np.stack(
            [b_qkv[sl], b_qkv[1024:][sl], b_qkv[2048:][sl]], axis=1
        )
        in_maps.append({
            "xh": xhp,
            "wc": np.ascontiguousarray(wcs).astype(ml_dtypes.bfloat16),
            "wout": np.ascontiguousarray(w_out[sl, :]).astype(
                ml_dtypes.bfloat16),
            "bqkv": np.ascontiguousarray(bq),
            "mtri": mtri,
            "ones": np.ones((128, 64), ml_dtypes.bfloat16),
            "onesr": np.ones((1, 64), np.float32),
        })
    return in_maps


_NC_CACHE = None


def kernel(x, w_qkv, b_qkv, w_out, b_out):
    global _NC_CACHE, LAST_RESULTS
    if _NC_CACHE is None:
        _NC_CACHE = build_nc()
    nc = _NC_CACHE

    in_maps = make_in_maps(x, w_qkv, b_qkv, w_out)

    res = run_bass_kernel_spmd(
        nc, in_maps, list(range(NCORES)),
        trace=bool(os.environ.get("BASS_TRACE")),
    )
    LAST_RESULTS = res

    acc = np.zeros((C, BT), np.float32)
    for out_map in res.results:
        # yh [p, tcx(8), m(8), t(512)] -> [m, p, tcx, t] -> [C, BT]
        yc = np.asarray(out_map["yh"]).reshape(128, NTC, 8, TC)
        acc += yc.transpose(2, 0, 1, 3).reshape(C, BT).astype(np.float32)
    y = acc.T + np.asarray(b_out, np.float32)[None, :]
    return y.reshape(B, T, C)
